# revision 26
# baseline (speedup 1.0000x reference)
"""Trainium2 Bass kernel for AdjacencyMatchingLoss.

Math: adj_score[b,e] = P[b,i_e,:] @ A @ P[b,j_e,:]  with A = (d_hw==1).
Let W[i,j] = sum_e w_e * 1[i_e=i] * 1[j_e=j]   (weighted pair histogram)
Then  total_adj = sum_ij W[i,j] * mean_b (P_b A P_b^T)[i,j]
               = (1/B) * sum_b < P_b^T W P_b , A >
Per core: shard edges (E/8), build W via one-hot matmuls on the
TensorEngine, compute C = sum_b P_b^T W P_b (layouts work out so no
transposes are ever needed), reduce <C, -A/8> and the local weight sum
to a 2-vector partial; host sums the 8 partials and divides.
"""

import os
import sys

import numpy as np

for _p in ("/opt/trn_rl_repo",):
    if os.path.isdir(_p) and _p not in sys.path:
        sys.path.insert(0, _p)

B, NL, NQ, E = 8, 128, 128, 50000
NCORES = 8
ESH = E // NCORES            # 6250 edges per core
CHUNKS = (ESH + 127) // 128  # 49
EPAD = CHUNKS * 128          # 6272

_BUILT = None


def _emit_body(nc, sp, pp, tensors):
    import concourse.mybir as mybir

    f32 = mybir.dt.float32
    bf16 = mybir.dt.bfloat16
    i32 = mybir.dt.int32
    EQ = mybir.AluOpType.is_equal
    MUL = mybir.AluOpType.mult
    ADD = mybir.AluOpType.add
    P_d, d_d, pr_d, w_d, o_d = tensors

    Pf = sp.tile([128, B * NQ], f32)
    Pb = sp.tile([128, B * NQ], bf16)
    dsb = sp.tile([128, NQ], i32)
    Asc = sp.tile([128, NQ], f32)
    prs = sp.tile([128, CHUNKS * 8], mybir.dt.int16)
    wT = sp.tile([128, CHUNKS], f32)
    idxI = sp.tile([128, CHUNKS], f32)
    idxJ = sp.tile([128, CHUNKS], f32)
    wNeg = sp.tile([128, CHUNKS], f32)
    iot = sp.tile([128, 128], bf16)
    OhJ = sp.tile([128, EPAD], bf16)
    OhIW = sp.tile([128, EPAD], bf16)
    Wsb = sp.tile([128, 128], bf16)
    Usb = sp.tile([128, B * NQ], bf16)
    prt = sp.tile([128, 2], f32)
    scr = sp.tile([128, NQ], f32)

    Wps = pp.tile([128, 128], f32)
    Up0 = pp.tile([128, 512], f32)
    Up1 = pp.tile([128, 512], f32)
    Cps = pp.tile([128, 128], f32)

    # ---- loads ----
    # pairs + weights first: they gate the critical one-hot chain.
    # Edge e lives at (partition p, chunk c) with e = p*CHUNKS + c, so each
    # partition reads a contiguous run (good DMA bursts).
    nc.sync.dma_start(
        out=prs[:].rearrange("p (c k) -> p c k", k=8),
        in_=pr_d.ap().rearrange("(p c) k -> p c k", p=128),
    )
    nc.sync.dma_start(out=wT[:], in_=w_d.ap().rearrange("(p c) -> p c", p=128))
    # split P across two queues, issued from the idle tensor/scalar
    # sequencers so the setups don't serialize behind sync's queue.
    # P is only needed late (U/C matmuls); d_hw at the very end.
    P_src = P_d.ap().rearrange("b l q -> l b q")
    Pf3 = Pf[:].rearrange("l (b q) -> l b q", q=NQ)
    nc.sync.dma_start(out=Pf3[:, 0:4, :], in_=P_src[:, 0:4, :])
    nc.sync.dma_start(out=Pf3[:, 4:8, :], in_=P_src[:, 4:8, :])
    nc.sync.dma_start(out=dsb[:], in_=d_d.ap())

    # ---- prep ----
    nc.gpsimd.iota(
        iot[:],
        pattern=[[1, 128]],
        base=0,
        channel_multiplier=0,
        allow_small_or_imprecise_dtypes=True,
    )
    prs3 = prs[:].rearrange("p (c k) -> p c k", k=8)
    nc.vector.tensor_copy(
        out=idxI[:].rearrange("p (c u) -> p c u", u=1), in_=prs3[:, :, 0:1]
    )
    nc.gpsimd.tensor_copy(
        out=idxJ[:].rearrange("p (c u) -> p c u", u=1), in_=prs3[:, :, 4:5]
    )
    # -w, used by the scalar-engine one-hot trick below
    nc.gpsimd.tensor_scalar(
        out=wNeg[:], in0=wT[:], scalar1=-1.0, scalar2=None, op0=MUL
    )
    # weight sum: off the critical tail, DVE is still cheap here
    nc.vector.tensor_reduce(
        out=prt[:, 1:2], in_=wT[:], axis=mybir.AxisListType.X, op=ADD
    )

    # ---- one-hots + W accumulation over chunks of 128 edges ----
    # Three engines build one-hot chunks in parallel, weighted by their
    # per-op speed: DVE (is_equal, ~94ns/op) takes most chunks, gpsimd
    # (same ops, ~3x slower) and ACT take the rest. ACT has no is_equal,
    # so it uses onehot = relu(1 - |i - idx|), and folds the edge weight
    # in via relu(w - w*t) = w*relu(1-t)  (valid since w >= 0).
    ACT_CHUNKS = {5, 15, 25, 35, 45}
    POOL_CHUNKS = {3, 8, 13, 18, 23, 28, 33, 38, 43}
    ABS = mybir.ActivationFunctionType.Abs
    RELU = mybir.ActivationFunctionType.Relu
    for c in range(CHUNKS):
        sl = slice(c * 128, (c + 1) * 128)
        if c == 40:
            # P f32->bf16 slipped into the one-hot stream (P arrived long
            # ago); ready just in time for the U matmuls at the tail.
            nc.vector.tensor_copy(out=Pb[:, 0:512], in_=Pf[:, 0:512])
        if c == 43:
            nc.gpsimd.tensor_copy(out=Pb[:, 512:1024], in_=Pf[:, 512:1024])
        if c in ACT_CHUNKS:
            tmpJ = sp.tile([128, 128], bf16, name=f"tmpJ{c}")
            tmpI = sp.tile([128, 128], bf16, name=f"tmpI{c}")
            nc.scalar.activation(
                out=tmpJ[:], in_=iot[:], func=ABS,
                bias=idxJ[:, c : c + 1], scale=-1.0,
            )
            nc.scalar.activation(
                out=OhJ[:, sl], in_=tmpJ[:], func=RELU, bias=1.0, scale=-1.0
            )
            nc.scalar.activation(
                out=tmpI[:], in_=iot[:], func=ABS,
                bias=idxI[:, c : c + 1], scale=-1.0,
            )
            nc.scalar.activation(
                out=OhIW[:, sl], in_=tmpI[:], func=RELU,
                bias=wT[:, c : c + 1], scale=wNeg[:, c : c + 1],
            )
        else:
            eng = nc.gpsimd if c in POOL_CHUNKS else nc.vector
            eng.tensor_scalar(
                out=OhJ[:, sl],
                in0=iot[:],
                scalar1=idxJ[:, c : c + 1],
                scalar2=None,
                op0=EQ,
            )
            eng.tensor_scalar(
                out=OhIW[:, sl],
                in0=iot[:],
                scalar1=idxI[:, c : c + 1],
                scalar2=wT[:, c : c + 1],
                op0=EQ,
                op1=MUL,
            )
        nc.tensor.matmul(
            Wps[:],
            lhsT=OhIW[:, sl],
            rhs=OhJ[:, sl],
            start=(c == 0),
            stop=(c == CHUNKS - 1),
        )
    nc.vector.tensor_copy(out=Wsb[:], in_=Wps[:])
    # A_scaled = -(1/8) * (d_hw == 1); folds sign+batch-mean. gpsimd is
    # done with its one-hot share by now; A is consumed at the tail only.
    nc.gpsimd.tensor_scalar(
        out=Asc[:], in0=dsb[:], scalar1=1, scalar2=-0.125, op0=EQ, op1=MUL
    )

    # ---- U = W^T P_b for all b (two 512-wide matmuls) ----
    nc.tensor.matmul(Up0[:], lhsT=Wsb[:], rhs=Pb[:, 0:512], start=True, stop=True)
    nc.tensor.matmul(Up1[:], lhsT=Wsb[:], rhs=Pb[:, 512:1024], start=True, stop=True)
    # PSUM->SBUF bf16 copies run in parallel on DVE and ACT
    nc.vector.tensor_copy(out=Usb[:, 0:512], in_=Up0[:])
    nc.scalar.copy(out=Usb[:, 512:1024], in_=Up1[:])

    # ---- C = sum_b P_b^T W P_b ----
    for b in range(B):
        sl = slice(b * 128, (b + 1) * 128)
        nc.tensor.matmul(
            Cps[:],
            lhsT=Usb[:, sl],
            rhs=Pb[:, sl],
            start=(b == 0),
            stop=(b == B - 1),
        )

    # ---- partials: [ <C, -A/8> , sum(w) ] ----
    # (tensor_tensor_reduce crashes on HW via the PJRT path; use a plain
    # multiply + free-axis reduce instead)
    nc.vector.tensor_tensor(out=scr[:], in0=Cps[:], in1=Asc[:], op=MUL)
    nc.vector.tensor_reduce(
        out=prt[:, 0:1], in_=scr[:], axis=mybir.AxisListType.X, op=ADD
    )
    # partition-dim reduction of prt [128,2] happens on the host, together
    # with the cross-core reduction
    nc.sync.dma_start(out=o_d.ap(), in_=prt[:])


def _build(reps=1):
    import concourse.bacc as bacc
    import concourse.mybir as mybir
    import concourse.tile as tile

    f32 = mybir.dt.float32
    i32 = mybir.dt.int32
    i16 = mybir.dt.int16

    nc = bacc.Bacc("TRN2", target_bir_lowering=False, debug=False, num_devices=NCORES)

    P_d = nc.dram_tensor("p_in", [B, NL, NQ], f32, kind="ExternalInput")
    d_d = nc.dram_tensor("d_in", [NQ, NQ], i32, kind="ExternalInput")
    pr_d = nc.dram_tensor("pairs_in", [EPAD, 8], i16, kind="ExternalInput")
    w_d = nc.dram_tensor("w_in", [EPAD], f32, kind="ExternalInput")
    o_d = nc.dram_tensor("out", [128, 2], f32, kind="ExternalOutput")

    with tile.TileContext(nc) as tc:
        with (
            tc.tile_pool(name="sbuf", bufs=1) as sp,
            tc.tile_pool(name="psum", bufs=1, space="PSUM") as pp,
        ):
            for _ in range(reps):
                _emit_body(nc, sp, pp, (P_d, d_d, pr_d, w_d, o_d))

    nc.compile()
    return nc


def _get_built():
    global _BUILT
    if _BUILT is None:
        _BUILT = _build()
    return _BUILT


def _shard_inputs(P, d_hw, circuit_edge_pairs, circuit_edge_weights):
    P = np.ascontiguousarray(np.asarray(P, dtype=np.float32))
    d_hw = np.ascontiguousarray(np.asarray(d_hw, dtype=np.int32))
    pairs = np.asarray(circuit_edge_pairs).astype(np.int64, copy=False)
    w = np.asarray(circuit_edge_weights, dtype=np.float32)

    pairs_pad = np.zeros((NCORES, EPAD, 2), dtype=np.int64)
    w_pad = np.zeros((NCORES, EPAD), dtype=np.float32)
    pairs_pad[:, :ESH] = pairs.reshape(NCORES, ESH, 2)
    w_pad[:, :ESH] = w.reshape(NCORES, ESH)
    pairs16 = pairs_pad.view(np.int16).reshape(NCORES, EPAD, 8)

    return [
        {
            "p_in": P,
            "d_in": d_hw,
            "pairs_in": np.ascontiguousarray(pairs16[i]),
            "w_in": np.ascontiguousarray(w_pad[i]),
        }
        for i in range(NCORES)
    ]


def _combine(results):
    parts = np.stack([np.asarray(results[i]["out"]) for i in range(NCORES)])
    numer = float(parts[:, :, 0].astype(np.float64).sum())
    wsum = float(parts[:, :, 1].astype(np.float64).sum())
    return np.float32(numer / max(wsum, 1e-8))


def kernel(P, d_hw, circuit_edge_pairs, circuit_edge_weights, _want_results=False):
    from concourse.bass_utils import run_bass_kernel_spmd

    nc = _get_built()
    in_maps = _shard_inputs(P, d_hw, circuit_edge_pairs, circuit_edge_weights)
    res = run_bass_kernel_spmd(nc, in_maps, core_ids=list(range(NCORES)))
    out = _combine(res.results)
    if _want_results:
        return out, res
    return out
